# revision 1
# baseline (speedup 1.0000x reference)
"""Trainium2 Bass kernel for nn_Classifier (attribute-sharded MLP heads).

Reference computation (B=64, C=1280, H=W=7, A=40):
    p   = h_swish(mean(x, axis=(2,3)))            # [B, C]
    h   = h_swish(einsum("bc,acd->bad", p, W1) + b1)
    out = sigmoid(einsum("bac,ac->ba", h, W2) + b2)  # [B, A]

Sharding: 8 cores, each owns A/8 = 5 attribute heads (W1/b1/W2/b2 shards);
x is replicated (pre-transposed on host to [C, B*49] so pooling lands in
the matmul-ready [c, b] orientation with zero on-chip transposes).

All large operands are cast to bf16 on host (measured absmax output error
~3e-5 vs fp32 reference; logits are tiny so sigmoid compresses further).
PSUM accumulation stays fp32.
"""

import sys

for _p in ("/opt/trn_rl_repo",):
    if _p not in sys.path:
        sys.path.insert(0, _p)

from contextlib import ExitStack

import numpy as np
import ml_dtypes

import concourse.bass as bass
import concourse.tile as tile
from concourse import bacc, mybir

# Problem constants (hardcoded per contract)
B = 64          # batch
C = 1280        # channels / features
S = 49          # spatial H*W
A = 40          # total attribute heads
NCORES = 8
AH = A // NCORES  # heads per core = 5
P = 128
KC = C // P       # 10 contraction chunks
NS = [(0, 512), (512, 512), (1024, 256)]  # psum n-chunks of C=1280

BF = mybir.dt.bfloat16
F32 = mybir.dt.float32
AF = mybir.ActivationFunctionType
ALU = mybir.AluOpType

_NC_CACHE = {}


def build_nc(reps=1):
    """Build the per-core Bass program (same program on all 8 cores).

    reps>1 unrolls the whole computation back-to-back (same inputs,
    same output) — used only for steady-state throughput benchmarking.
    """
    nc = bacc.Bacc("TRN2", target_bir_lowering=False, name="attr_mlp")

    xT = nc.dram_tensor("xT", [C, B * S], BF, kind="ExternalInput")
    w1 = nc.dram_tensor("w1", [AH, C, C], BF, kind="ExternalInput")
    b1 = nc.dram_tensor("b1", [AH * C], BF, kind="ExternalInput")
    # W2 broadcast with head-major layout: row a*B+b holds W2[a, :]
    w2b = nc.dram_tensor("w2b", [AH * B, C], BF, kind="ExternalInput")
    b2b = nc.dram_tensor("b2b", [AH * B], F32, kind="ExternalInput")
    # output in [head, batch] layout; host transposes back
    out = nc.dram_tensor("out", [AH, B], F32, kind="ExternalOutput")

    with tile.TileContext(nc) as tc, ExitStack() as ctx:
        const = ctx.enter_context(tc.tile_pool(name="const", bufs=1))
        st = ctx.enter_context(tc.tile_pool(name="st", bufs=2))
        xp = ctx.enter_context(tc.tile_pool(name="xp", bufs=5))
        wg = ctx.enter_context(tc.tile_pool(name="wg", bufs=1))
        wp = ctx.enter_context(tc.tile_pool(name="wp", bufs=10))
        sp = ctx.enter_context(tc.tile_pool(name="sp", bufs=3))
        pp = ctx.enter_context(tc.tile_pool(name="pp", bufs=2, space="PSUM"))

        # head groups: pairs share the 128 partitions (64 each) via PE
        # column tiling; the odd head runs solo on 64 partitions, FIRST,
        # so the group finishing last (exposed tail) is a full-width pair.
        groups = [(4,), (0, 1), (2, 3)]

        # --- constants (loaded once) ---
        ones = const.tile([1, B], BF)
        nc.vector.memset(ones, 1.0)
        half = const.tile([P, 1], F32)  # bias=0.5 for the h_swish Relu
        nc.vector.memset(half, 0.5)
        b1_sb = const.tile([1, AH * C], BF)
        # constants ride the ACT HWDGE ring so they can't head-of-line
        # block the x/W1 stream on the SP ring
        nc.scalar.dma_start(b1_sb, b1[None, :])
        w2_g, b2_g = [], []
        for g, hs in enumerate(groups):
            pn = B * len(hs)
            r0 = hs[0] * B
            w2t = const.tile([pn, C], BF, tag=f"w2_{g}")
            nc.scalar.dma_start(w2t, w2b[r0:r0 + pn, :])
            b2t = const.tile([pn, 1], F32, tag=f"b2_{g}")
            nc.scalar.dma_start(b2t, b2b[r0:r0 + pn, None])
            w2_g.append(w2t)
            b2_g.append(b2t)

        for _rep in range(reps):
            # pT[ci, k, b] = h_swish(mean(x))[b, 128*k + ci]  (matmul lhsT)
            pT = st.tile([P, KC, B], BF, tag="pT")

            # --- stage 1: pooling + h_swish -> pT ---
            # pT holds 49*p*clip(p/6+1/2,0,1); the 1/49 is pre-folded into
            # W1 on the host, so GEMM1 still computes p @ W1.
            for k in range(KC):
                xt = xp.tile([P, B * S], BF, tag="xt")
                nc.sync.dma_start(xt, xT[k * P:(k + 1) * P, :])
                sums = sp.tile([P, B], F32, tag="sums")
                nc.vector.reduce_sum(
                    sums, xt.rearrange("p (b s) -> p b s", s=S),
                    axis=mybir.AxisListType.X,
                )
                t1 = sp.tile([P, B], F32, tag="t1")
                nc.scalar.activation(
                    t1, sums, AF.Relu, bias=half, scale=1.0 / (6.0 * 49.0)
                )
                nc.vector.scalar_tensor_tensor(
                    pT[:, k, :], t1, 1.0, sums, ALU.min, ALU.mult
                )

            # --- stage 2: per-head-group GEMM + h_swish + dot(W2) ---
            for g, hs in enumerate(groups):
                pn = B * len(hs)
                nh = len(hs)
                last = g == len(groups) - 1
                # heads of a pair live on disjoint 64-partition halves of
                # the same PSUM banks (PE column tiling). One psum tile
                # per n-chunk so Tile doesn't serialize cross-chunk
                # PSUM readers.
                pss = []
                for ni, (n0, nn) in enumerate(NS):
                    pst = pp.tile([P, nn], F32, tag=f"ps{ni}", name=f"ps{ni}")
                    pss.append(pst)

                def bias_mms():
                    # b1 via K=1 outer product; issued right after the
                    # k=0 matmuls so they don't trail the weight stream
                    for j, a in enumerate(hs):
                        tp = (0, 64 * j) if j else None
                        for ni, (n0, nn) in enumerate(NS):
                            nc.tensor.matmul(
                                pss[ni][64 * j:64 * j + B, :], ones,
                                b1_sb[:, a * C + n0:a * C + n0 + nn],
                                start=False, stop=False, tile_position=tp,
                                skip_group_check=True,
                            )

                if not last:
                    # one whole-group weight DMA (3.3/6.6 MB, max DMA
                    # efficiency; arrival granularity is irrelevant off
                    # the critical tail)
                    w1g = wg.tile([P, nh * KC, C], BF, tag=f"w1g{g}",
                                  name=f"w1g{g}")
                    src = w1[hs[0]:hs[0] + nh].rearrange(
                        "a (ko p) d -> p (a ko) d", p=P)
                    nc.sync.dma_start(w1g, src)
                    for k in range(KC):
                        for j, a in enumerate(hs):
                            tp = (0, 64 * j) if j else None
                            for ni, (n0, nn) in enumerate(NS):
                                nc.tensor.matmul(
                                    pss[ni][64 * j:64 * j + B, :],
                                    pT[:, k, :],
                                    w1g[:, j * KC + k, n0:n0 + nn],
                                    start=(k == 0), stop=(k == KC - 1),
                                    tile_position=tp,
                                    skip_group_check=True,
                                )
                        if k == 0:
                            bias_mms()
                else:
                    # column-major weight stream: n-chunk ni's
                    # accumulation closes at (ni+1)/3 of this group's
                    # stream, so its evacuation overlaps the remaining
                    # stream; only the last (256-wide) chunk is a tail.
                    # DMAs batch KH k-chunks (640/320 KB) to stay off the
                    # HWDGE descriptor-generation floor.
                    KH = 5
                    for ni, (n0, nn) in enumerate(NS):
                        for j, a in enumerate(hs):
                            tp = (0, 64 * j) if j else None
                            for kh in range(KC // KH):
                                w1kt = wp.tile([P, KH, 512], BF, tag="w1kt")
                                src = w1[a, kh * KH * P:(kh + 1) * KH * P,
                                         n0:n0 + nn].rearrange(
                                    "(ko p) d -> p ko d", p=P)
                                nc.sync.dma_start(w1kt[:, :, :nn], src)
                                for ko in range(KH):
                                    k = kh * KH + ko
                                    nc.tensor.matmul(
                                        pss[ni][64 * j:64 * j + B, :],
                                        pT[:, k, :], w1kt[:, ko, :nn],
                                        start=(k == 0), stop=(k == KC - 1),
                                        tile_position=tp,
                                        skip_group_check=True,
                                    )
                                    if k == 0:
                                        nc.tensor.matmul(
                                            pss[ni][64 * j:64 * j + B, :],
                                            ones,
                                            b1_sb[:, a * C + n0:
                                                  a * C + n0 + nn],
                                            start=False, stop=False,
                                            tile_position=tp,
                                            skip_group_check=True,
                                        )
                # evacuation per n-chunk; chains pipeline across chunks:
                #   t1h = Relu(z/6 + 1/2)            [ACT, psum read]
                #   t2w = min(t1h, 1) * w2           [DVE stt]
                #   scr = z * t2w; rpart = sum(scr)  [DVE stt, psum read]
                rpart = st.tile([P, len(NS)], F32, tag="rpart")
                for ni, (n0, nn) in enumerate(NS):
                    zs = pss[ni][:pn]
                    t1h = sp.tile([P, 512], F32, tag="t1h")
                    nc.scalar.activation(
                        t1h[:pn, :nn], zs, AF.Relu,
                        bias=half[:pn], scale=1.0 / 6.0,
                    )
                    t2w = sp.tile([P, 512], BF, tag="t2w")
                    nc.vector.scalar_tensor_tensor(
                        t2w[:pn, :nn], t1h[:pn, :nn], 1.0,
                        w2_g[g][:, n0:n0 + nn], ALU.min, ALU.mult,
                    )
                    scr = sp.tile([P, 512], F32, tag="scr")
                    nc.vector.scalar_tensor_tensor(
                        scr[:pn, :nn], zs, 1.0, t2w[:pn, :nn],
                        ALU.mult, ALU.mult,
                        accum_out=rpart[:pn, ni:ni + 1],
                    )
                rlog = st.tile([P, 1], F32, tag="rlog")
                nc.vector.reduce_sum(
                    rlog[:pn], rpart[:pn, :], axis=mybir.AxisListType.X
                )
                # sigmoid with fused +b2 (per-partition bias)
                osb = st.tile([P, 1], F32, tag="osb")
                nc.scalar.activation(
                    osb[:pn], rlog[:pn], AF.Sigmoid, bias=b2_g[g]
                )
                dst = out[hs[0]:hs[0] + len(hs), :].rearrange(
                    "h b -> (h b)")[:, None]
                if last:
                    # nothing left on the HWDGE rings to block, and HWDGE
                    # latency (~0.6us) beats SWDGE (~2us) on the tail
                    nc.sync.dma_start(dst, osb[:pn])
                else:
                    # SWDGE store: keeps the tiny result write off the
                    # HWDGE rings so it can't head-of-line-block weights
                    nc.gpsimd.dma_start(dst, osb[:pn])

    nc.compile()
    return nc


def get_nc(reps=1):
    if reps not in _NC_CACHE:
        _NC_CACHE[reps] = build_nc(reps)
    return _NC_CACHE[reps]


def make_in_maps(x, W1, b1, W2, b2):
    bf = ml_dtypes.bfloat16
    x = np.asarray(x, dtype=np.float32)
    W1 = np.asarray(W1, dtype=np.float32)
    b1 = np.asarray(b1, dtype=np.float32)
    W2 = np.asarray(W2, dtype=np.float32)
    b2 = np.asarray(b2, dtype=np.float32)

    # [B, C, H, W] -> [C, B*S], replicated to all cores
    xT = np.ascontiguousarray(
        x.reshape(B, C, S).transpose(1, 0, 2)
    ).reshape(C, B * S).astype(bf)

    in_maps = []
    for core in range(NCORES):
        a0 = core * AH
        w2s = W2[a0:a0 + AH]  # [AH, C]
        in_maps.append({
            "xT": xT,
            # 1/49 of the mean pooling is folded into W1 (pT carries 49*p)
            "w1": np.ascontiguousarray(W1[a0:a0 + AH] * (1.0 / 49.0)).astype(bf),
            "b1": np.ascontiguousarray(b1[a0:a0 + AH]).reshape(AH * C).astype(bf),
            # row a*B+b holds W2[a, :]
            "w2b": np.ascontiguousarray(
                np.broadcast_to(w2s[:, None, :], (AH, B, C)).reshape(AH * B, C)
            ).astype(bf),
            "b2b": np.ascontiguousarray(
                np.broadcast_to(b2[a0:a0 + AH, None], (AH, B)).reshape(AH * B)
            ).astype(np.float32),
        })
    return in_maps


def kernel(x, W1, b1, W2, b2, _trace=False, _tmpdir=None):
    from concourse.bass_utils import run_bass_kernel_spmd

    nc = get_nc()
    in_maps = make_in_maps(x, W1, b1, W2, b2)
    res = run_bass_kernel_spmd(
        nc, in_maps, core_ids=list(range(NCORES)),
        trace=_trace, tmpdir=_tmpdir,
    )
    outs = [np.asarray(res.results[c]["out"], dtype=np.float32).T
            for c in range(NCORES)]  # each [B, AH]
    full = np.concatenate(outs, axis=1)  # [B, A]
    if _trace:
        return full, res
    return full



# revision 7
# speedup vs baseline: 1.3834x; 1.3834x over previous
"""Trainium2 Bass kernel for nn_Classifier (attribute-sharded MLP heads).

Reference computation (B=64, C=1280, H=W=7, A=40):
    p   = h_swish(mean(x, axis=(2,3)))            # [B, C]
    h   = h_swish(einsum("bc,acd->bad", p, W1) + b1)
    out = sigmoid(einsum("bac,ac->ba", h, W2) + b2)  # [B, A]

Sharding: 8 cores, each owns A/8 = 5 attribute heads.

v2 design (from NTFF trace analysis of v1):
- Everything large rides fp8 E3M4 (4-bit mantissa) with power-of-2 scale
  folding so values sit in the normal range:
    x  -> e3m4 (randn, rms 1: in range natively)
    pT = 24.5 * p            (pool stage output, e3m4)
    W1' = W1 * 256/49        (e3m4)  => psum z' = 128 * z
    b1' = b1 * 128           (bf16 bias matmul)
    W2' = W2 * 64            (e3m4)
  evac: gate = min(Relu(z'/768 + .5), 1); t2w = gate*W2'; scr = (z'/8192)*t2w
  => scr = h_swish(z) * W2 exactly.
- W1 is pre-transposed ON HOST into the exact stream layout, so every
  weight DMA is a plain contiguous 2D copy (v1 lost ~30us to HWDGE
  descriptor generation for 3-level gather patterns).
- Pooling (the v1 serial bottleneck: 34.6us on DVE at 1x) is split
  DVE/GPSIMD by chunk.
- GEMM phase 1 runs heads {4,0,1} k-major interleaved (3 live PSUM
  groups) so pooling completion doesn't serialize whole head-groups;
  the last pair {2,3} streams n-major so its PSUM evacuation overlaps
  the remaining weight stream.
- No SWDGE stores (v1's gpsimd stores cost ~10us + epilogue waits);
  early groups store via the scalar-ring HWDGE, the last via sync.
"""

import sys

for _p in ("/opt/trn_rl_repo",):
    if _p not in sys.path:
        sys.path.insert(0, _p)

from contextlib import ExitStack

import numpy as np
import ml_dtypes

import concourse.bass as bass
import concourse.tile as tile
from concourse import bacc, mybir

# Problem constants (hardcoded per contract)
B = 64          # batch
C = 1280        # channels / features
S = 49          # spatial H*W
A = 40          # total attribute heads
NCORES = 8
AH = A // NCORES  # heads per core = 5
P = 128
KC = C // P       # 10 contraction chunks
NS = [(0, 512), (512, 512), (1024, 256)]  # psum n-chunks of C=1280

BF = mybir.dt.bfloat16
F8 = mybir.dt.float8e3
F32 = mybir.dt.float32
AF = mybir.ActivationFunctionType
ALU = mybir.AluOpType

# scale folding constants
SC_W1 = 256.0 / 49.0   # host W1 multiplier
SC_B1 = 128.0          # host b1 multiplier
SC_W2 = 64.0           # host W2 multiplier
T1_SCALE = 1.0 / (6.0 * 49.0 * 2.0)   # stage-1 gate: Relu(sums*s + 0.25)
T1H_SCALE = 1.0 / (6.0 * 128.0)       # evac gate: Relu(z'*s + 0.5)
SCR_SCALE = 1.0 / 8192.0              # z' * t2w descale (2^-7 * 2^-6)

# pooling chunk ownership: DVE is ~116 G elem/s at 1x; gpsimd takes the
# tail chunks (which arrive last anyway)
GP_CHUNKS = (6, 7, 8, 9)

PH1 = (4, 0, 1)   # phase-1 heads, k-major interleaved (4 solo + pair 0,1)
PH2 = (2, 3)      # phase-2 pair, n-major stream

_NC_CACHE = {}


def build_nc():
    nc = bacc.Bacc("TRN2", target_bir_lowering=False, name="attr_mlp")

    xT = nc.dram_tensor("xT", [C, B * S], F8, kind="ExternalInput")
    # phase-1 weight stream, k-major: [P, KC, 3 heads, C]
    wa = nc.dram_tensor("wa", [P, KC, 3, C], F8, kind="ExternalInput")
    # phase-2 weight stream, n-major sections: [P, KC, 2, nn] per section
    wb0 = nc.dram_tensor("wb0", [P, KC, 2, 512], F8, kind="ExternalInput")
    wb1 = nc.dram_tensor("wb1", [P, KC, 2, 512], F8, kind="ExternalInput")
    wb2 = nc.dram_tensor("wb2", [P, KC, 2, 256], F8, kind="ExternalInput")
    b1 = nc.dram_tensor("b1", [AH * C], BF, kind="ExternalInput")
    # W2 broadcast with head-major layout: row a*B+b holds W2[a, :] * 64
    w2b = nc.dram_tensor("w2b", [AH * B, C], F8, kind="ExternalInput")
    b2b = nc.dram_tensor("b2b", [AH * B], F32, kind="ExternalInput")
    # output in [head, batch] layout; host transposes back
    out = nc.dram_tensor("out", [AH, B], F32, kind="ExternalOutput")

    # evac groups: (group-id, heads, psum-tag-base)
    # phase1 groups: solo head 4 (pn=64), pair (0,1); phase2 pair (2,3)
    g_solo = (4,)
    g_a = (0, 1)
    g_b = (2, 3)

    with tile.TileContext(nc) as tc, ExitStack() as ctx:
        const = ctx.enter_context(tc.tile_pool(name="const", bufs=1))
        st = ctx.enter_context(tc.tile_pool(name="st", bufs=1))
        xp = ctx.enter_context(tc.tile_pool(name="xp", bufs=5))
        wga = ctx.enter_context(tc.tile_pool(name="wga", bufs=1))
        wgb = ctx.enter_context(tc.tile_pool(name="wgb", bufs=1))
        sp = ctx.enter_context(tc.tile_pool(name="sp", bufs=2))
        ep = ctx.enter_context(tc.tile_pool(name="ep", bufs=2))
        pp = ctx.enter_context(tc.tile_pool(name="pp", bufs=1, space="PSUM"))

        # --- constants ---
        ones = const.tile([1, B], BF)
        nc.gpsimd.memset(ones, 1.0)
        half = const.tile([P, 1], F32)
        nc.gpsimd.memset(half, 0.5)
        quart = const.tile([P, 1], F32)
        nc.gpsimd.memset(quart, 0.25)
        b1_sb = const.tile([1, AH * C], BF)
        nc.scalar.dma_start(b1_sb, b1[None, :])
        w2_g, b2_g = {}, {}
        for hs in (g_solo, g_a, g_b):
            pn = B * len(hs)
            r0 = hs[0] * B
            w2t = const.tile([pn, C], F8, tag=f"w2_{hs[0]}")
            nc.scalar.dma_start(w2t, w2b[r0:r0 + pn, :])
            b2t = const.tile([pn, 1], F32, tag=f"b2_{hs[0]}")
            nc.scalar.dma_start(b2t, b2b[r0:r0 + pn, None])
            w2_g[hs] = w2t
            b2_g[hs] = b2t

        # --- input DMAs all issued up front, in consumption order ---
        # x: 5 pair-chunk DMAs
        xts = []
        for kp in range(KC // 2):
            xt = xp.tile([P, 2, B * S], F8, tag="xt")
            src = xT[kp * 2 * P:(kp + 1) * 2 * P, :].rearrange(
                "(two p) f -> p two f", p=P)
            nc.sync.dma_start(xt, src)
            xts.append(xt)
        # phase-1 weights: one DMA per k (plain 2D contiguous)
        wat = wga.tile([P, KC, 3, C], F8)
        for k in range(KC):
            nc.sync.dma_start(wat[:, k], wa[:, k])
        # phase-2 weights: 2 DMAs per 512-section, 1 for the 256 tail
        wbt0 = wgb.tile([P, KC, 2, 512], F8, tag="wb0", name="wb0")
        wbt1 = wgb.tile([P, KC, 2, 512], F8, tag="wb1", name="wb1")
        wbt2 = wgb.tile([P, KC, 2, 256], F8, tag="wb2", name="wb2")
        KH = 5
        for ni, (wt, wd) in enumerate(((wbt0, wb0), (wbt1, wb1), (wbt2, wb2))):
            for kh in range(KC // KH):
                nc.sync.dma_start(
                    wt[:, kh * KH:(kh + 1) * KH], wd[:, kh * KH:(kh + 1) * KH])

        # --- stage 1: pooling + h_swish gate -> pT (fp8, 24.5*p) ---
        pT = st.tile([P, KC, B], F8, tag="pT")
        for k in range(KC):
            xsl = xts[k // 2][:, k % 2, :].rearrange("p (b s) -> p b s", s=S)
            sums = sp.tile([P, B], F32, tag="sums")
            nc.vector.reduce_sum(sums, xsl, axis=mybir.AxisListType.X)
            t1 = sp.tile([P, B], F32, tag="t1")
            nc.scalar.activation(t1, sums, AF.Relu, bias=quart, scale=T1_SCALE)
            nc.vector.scalar_tensor_tensor(
                pT[:, k, :], t1, 0.5, sums, ALU.min, ALU.mult)

        # --- PSUM tiles ---
        # phase1: 6 live tiles; phase2 reuses the solo group's tags
        def psum_tiles(base):
            ts = []
            for ni, (n0, nn) in enumerate(NS):
                ts.append(pp.tile([P, nn], F32, tag=f"{base}{ni}",
                                  name=f"{base}{ni}"))
            return ts

        ps_solo = psum_tiles("pss")
        ps_a = psum_tiles("psa")

        def bias_mm(ps, j, a, n0, nn):
            tp = (0, 64 * j) if j else None
            nc.tensor.matmul(
                ps[64 * j:64 * j + B, :], ones,
                b1_sb[:, a * C + n0:a * C + n0 + nn],
                start=False, stop=False, tile_position=tp,
                skip_group_check=True)

        # --- phase 1: heads (4 | 0,1) k-major interleaved ---
        for k in range(KC):
            for ni, (n0, nn) in enumerate(NS):
                # solo head 4: psum partitions 0-63, PE cols 0-63
                nc.tensor.matmul(
                    ps_solo[ni][:B, :], pT[:, k, :], wat[:, k, 0, n0:n0 + nn],
                    start=(k == 0), stop=(k == KC - 1),
                    skip_group_check=True)
            for j in (0, 1):  # pair heads 0,1
                tp = (0, 64 * j) if j else None
                for ni, (n0, nn) in enumerate(NS):
                    nc.tensor.matmul(
                        ps_a[ni][64 * j:64 * j + B, :], pT[:, k, :],
                        wat[:, k, 1 + j, n0:n0 + nn],
                        start=(k == 0), stop=(k == KC - 1),
                        tile_position=tp, skip_group_check=True)
            if k == 0:
                for ni, (n0, nn) in enumerate(NS):
                    bias_mm(ps_solo[ni], 0, 4, n0, nn)
                for j, a in enumerate(g_a):
                    for ni, (n0, nn) in enumerate(NS):
                        bias_mm(ps_a[ni], j, a, n0, nn)

        # --- evacuation helper ---
        def evac(hs, pss, store_ring):
            pn = B * len(hs)
            rpart = st.tile([P, len(NS)], F32, tag=f"rp{hs[0]}")
            for ni, (n0, nn) in enumerate(NS):
                evac_chunk(hs, pss, ni, n0, nn, rpart)
            evac_fin(hs, rpart, store_ring)

        def evac_chunk(hs, pss, ni, n0, nn, rpart):
            pn = B * len(hs)
            zs = pss[ni][:pn]
            t1h = ep.tile([P, 512], F32, tag="t1h")
            nc.scalar.activation(
                t1h[:pn, :nn], zs, AF.Relu, bias=half[:pn], scale=T1H_SCALE)
            t2w = ep.tile([P, 512], BF, tag="t2w")
            nc.vector.scalar_tensor_tensor(
                t2w[:pn, :nn], t1h[:pn, :nn], 1.0,
                w2_g[hs][:, n0:n0 + nn], ALU.min, ALU.mult)
            scr = ep.tile([P, 512], F32, tag="scr")
            nc.vector.scalar_tensor_tensor(
                scr[:pn, :nn], zs, SCR_SCALE, t2w[:pn, :nn],
                ALU.mult, ALU.mult, accum_out=rpart[:pn, ni:ni + 1])

        def evac_fin(hs, rpart, store_ring):
            pn = B * len(hs)
            rlog = st.tile([P, 1], F32, tag=f"rl{hs[0]}")
            nc.vector.reduce_sum(rlog[:pn], rpart[:pn, :],
                                 axis=mybir.AxisListType.X)
            osb = st.tile([P, 1], F32, tag=f"os{hs[0]}")
            nc.scalar.activation(osb[:pn], rlog[:pn], AF.Sigmoid,
                                 bias=b2_g[hs])
            dst = out[hs[0]:hs[0] + len(hs), :].rearrange(
                "h b -> (h b)")[:, None]
            store_ring.dma_start(dst, osb[:pn])

        evac(g_solo, ps_solo, nc.scalar)
        evac(g_a, ps_a, nc.scalar)

        # --- phase 2: pair (2,3), n-major stream; psum reuses solo tags ---
        ps_b = psum_tiles("pss")
        rpart_b = st.tile([P, len(NS)], F32, tag="rp2")
        for ni, (n0, nn) in enumerate(NS):
            wt = (wbt0, wbt1, wbt2)[ni]
            for j in (0, 1):
                tp = (0, 64 * j) if j else None
                for k in range(KC):
                    nc.tensor.matmul(
                        ps_b[ni][64 * j:64 * j + B, :], pT[:, k, :],
                        wt[:, k, j, :nn],
                        start=(k == 0), stop=(k == KC - 1),
                        tile_position=tp, skip_group_check=True)
                    if k == 0:
                        bias_mm(ps_b[ni], j, g_b[j], n0, nn)
            evac_chunk(g_b, ps_b, ni, n0, nn, rpart_b)

        evac_fin(g_b, rpart_b, nc.sync)

    nc.compile()
    return nc


def get_nc():
    if "nc" not in _NC_CACHE:
        _NC_CACHE["nc"] = build_nc()
    return _NC_CACHE["nc"]


def make_in_maps(x, W1, b1, W2, b2):
    f8 = ml_dtypes.float8_e3m4
    bf = ml_dtypes.bfloat16
    x = np.asarray(x, dtype=np.float32)
    W1 = np.asarray(W1, dtype=np.float32)
    b1 = np.asarray(b1, dtype=np.float32)
    W2 = np.asarray(W2, dtype=np.float32)
    b2 = np.asarray(b2, dtype=np.float32)

    # [B, C, H, W] -> [C, B*S], replicated to all cores
    xT = np.ascontiguousarray(
        x.reshape(B, C, S).transpose(1, 0, 2)
    ).reshape(C, B * S).astype(f8)

    in_maps = []
    for core in range(NCORES):
        a0 = core * AH
        # per-head k-chunked layout: wh[a][k, p, d] = W1[a0+a][k*128+p, d]
        whs = (W1[a0:a0 + AH] * SC_W1).reshape(AH, KC, P, C)
        # phase-1 stream [P, KC, 3, C], heads (4, 0, 1)
        wa = np.empty((P, KC, 3, C), np.float32)
        for jj, a in enumerate(PH1):
            wa[:, :, jj, :] = whs[a].transpose(1, 0, 2)
        # phase-2 streams [P, KC, 2, nn] per n-section, heads (2, 3)
        wbs = []
        for n0, nn in NS:
            wbn = np.empty((P, KC, 2, nn), np.float32)
            for jj, a in enumerate(PH2):
                wbn[:, :, jj, :] = whs[a][:, :, n0:n0 + nn].transpose(1, 0, 2)
            wbs.append(wbn)
        w2s = W2[a0:a0 + AH] * SC_W2  # [AH, C]
        in_maps.append({
            "xT": xT,
            "wa": np.ascontiguousarray(wa).astype(f8),
            "wb0": np.ascontiguousarray(wbs[0]).astype(f8),
            "wb1": np.ascontiguousarray(wbs[1]).astype(f8),
            "wb2": np.ascontiguousarray(wbs[2]).astype(f8),
            "b1": np.ascontiguousarray(b1[a0:a0 + AH] * SC_B1
                                       ).reshape(AH * C).astype(bf),
            "w2b": np.ascontiguousarray(
                np.broadcast_to(w2s[:, None, :], (AH, B, C)).reshape(AH * B, C)
            ).astype(f8),
            "b2b": np.ascontiguousarray(
                np.broadcast_to(b2[a0:a0 + AH, None], (AH, B)).reshape(AH * B)
            ).astype(np.float32),
        })
    return in_maps


def kernel(x, W1, b1, W2, b2, _trace=False, _tmpdir=None):
    from concourse.bass_utils import run_bass_kernel_spmd

    nc = get_nc()
    in_maps = make_in_maps(x, W1, b1, W2, b2)
    res = run_bass_kernel_spmd(
        nc, in_maps, core_ids=list(range(NCORES)),
        trace=_trace, tmpdir=_tmpdir,
    )
    outs = [np.asarray(res.results[c]["out"], dtype=np.float32).T
            for c in range(NCORES)]  # each [B, AH]
    full = np.concatenate(outs, axis=1)  # [B, A]
    if _trace:
        return full, res
    return full


# revision 21
# speedup vs baseline: 1.6657x; 1.2041x over previous
"""Trainium2 Bass kernel for nn_Classifier (attribute-sharded MLP heads).

Reference computation (B=64, C=1280, H=W=7, A=40):
    p   = h_swish(mean(x, axis=(2,3)))            # [B, C]
    h   = h_swish(einsum("bc,acd->bad", p, W1) + b1)
    out = sigmoid(einsum("bac,ac->ba", h, W2) + b2)  # [B, A]

Sharding: 8 cores, each owns A/8 = 5 attribute heads.

v2 design (from NTFF trace analysis of v1):
- Everything large rides fp8 E3M4 (4-bit mantissa) with power-of-2 scale
  folding so values sit in the normal range:
    x  -> e3m4 (randn, rms 1: in range natively)
    pT = 24.5 * p            (pool stage output, e3m4)
    W1' = W1 * 256/49        (e3m4)  => psum z' = 128 * z
    b1' = b1 * 128           (bf16 bias matmul)
    W2' = W2 * 64            (e3m4)
  evac: gate = min(Relu(z'/768 + .5), 1); t2w = gate*W2'; scr = (z'/8192)*t2w
  => scr = h_swish(z) * W2 exactly.
- W1 is pre-transposed ON HOST into the exact stream layout, so every
  weight DMA is a plain contiguous 2D copy (v1 lost ~30us to HWDGE
  descriptor generation for 3-level gather patterns).
- Pooling (the v1 serial bottleneck: 34.6us on DVE at 1x) is split
  DVE/GPSIMD by chunk.
- GEMM phase 1 runs heads {4,0,1} k-major interleaved (3 live PSUM
  groups) so pooling completion doesn't serialize whole head-groups;
  the last pair {2,3} streams n-major so its PSUM evacuation overlaps
  the remaining weight stream.
- No SWDGE stores (v1's gpsimd stores cost ~10us + epilogue waits);
  early groups store via the scalar-ring HWDGE, the last via sync.
"""

import sys

for _p in ("/opt/trn_rl_repo",):
    if _p not in sys.path:
        sys.path.insert(0, _p)

from contextlib import ExitStack

import numpy as np
import ml_dtypes

import concourse.bass as bass
import concourse.tile as tile
from concourse import bacc, mybir

# Problem constants (hardcoded per contract)
B = 64          # batch
C = 1280        # channels / features
S = 49          # spatial H*W
SP50 = 50       # S zero-padded to 50 so the gpsimd pairwise add halves evenly
A = 40          # total attribute heads
NCORES = 8
AH = A // NCORES  # heads per core = 5
P = 128
KC = C // P       # 10 contraction chunks
NS = [(0, 512), (512, 512), (1024, 256)]  # psum n-chunks of C=1280

BF = mybir.dt.bfloat16
F8 = mybir.dt.float8e3
F32 = mybir.dt.float32
AF = mybir.ActivationFunctionType
ALU = mybir.AluOpType

# scale folding constants
SC_W1 = 256.0 / 49.0   # host W1 multiplier
SC_B1 = 128.0          # host b1 multiplier
SC_W2 = 64.0           # host W2 multiplier
T1_SCALE = 1.0 / (6.0 * 49.0 * 2.0)   # stage-1 gate: Relu(sums*s + 0.25)
T1H_SCALE = 1.0 / (6.0 * 128.0)       # evac gate: Relu(z'*s + 0.5)
SCR_SCALE = 1.0 / 8192.0              # z' * t2w descale (2^-7 * 2^-6)

PH1 = (4, 0, 1)   # phase-1 heads, k-major interleaved (4 solo + pair 0,1)
PH2 = (2, 3)      # phase-2 pair, n-major stream

# const-block order: column-block cb of w2c/b2c/osb32 belongs to group
# GORDER[cb]; output flat index = 128*cb + within (heads 0,1 | 2,3 | 4+pad)
GORDER = ((0, 1), (2, 3), (4,))

_NC_CACHE = {}


def build_nc():
    nc = bacc.Bacc("TRN2", target_bir_lowering=False, name="attr_mlp")

    xT = nc.dram_tensor("xT", [C, B * SP50], F8, kind="ExternalInput")
    # phase-1 weight stream, k-major: [P, KC, 3 heads, C]
    wa = nc.dram_tensor("wa", [P, KC, 3, C], F8, kind="ExternalInput")
    # phase-2 weight stream, n-major sections: [P, KC, 2, nn] per section
    wb0 = nc.dram_tensor("wb0", [P, KC, 2, 512], F8, kind="ExternalInput")
    wb1 = nc.dram_tensor("wb1", [P, KC, 2, 512], F8, kind="ExternalInput")
    wb2 = nc.dram_tensor("wb2", [P, KC, 2, 256], F8, kind="ExternalInput")
    b1 = nc.dram_tensor("b1", [AH * C], BF, kind="ExternalInput")
    # W2 per column-block cb (GORDER), rows = 64*head_within + b
    w2c = nc.dram_tensor("w2c", [P, 3, C], F8, kind="ExternalInput")
    b2c = nc.dram_tensor("b2c", [P, 3], F32, kind="ExternalInput")
    # output flat (cb, head_within, batch), row 5 of 6 is padding
    out = nc.dram_tensor("out", [6 * B], F32, kind="ExternalOutput")

    # evac groups: (group-id, heads, psum-tag-base)
    # phase1 groups: solo head 4 (pn=64), pair (0,1); phase2 pair (2,3)
    g_solo = (4,)
    g_a = (0, 1)
    g_b = (2, 3)

    with tile.TileContext(nc) as tc, ExitStack() as ctx:
        const = ctx.enter_context(tc.tile_pool(name="const", bufs=1))
        st = ctx.enter_context(tc.tile_pool(name="st", bufs=1))
        xp = ctx.enter_context(tc.tile_pool(name="xp", bufs=5))
        wga = ctx.enter_context(tc.tile_pool(name="wga", bufs=1))
        wgb = ctx.enter_context(tc.tile_pool(name="wgb", bufs=1))
        sp = ctx.enter_context(tc.tile_pool(name="sp", bufs=2))
        ep = ctx.enter_context(tc.tile_pool(name="ep", bufs=2))
        pp = ctx.enter_context(tc.tile_pool(name="pp", bufs=1, space="PSUM"))

        # --- constants ---
        ones = const.tile([1, B], BF)
        nc.gpsimd.memset(ones, 1.0)
        half = const.tile([P, 1], F32)
        nc.gpsimd.memset(half, 0.5)
        quart = const.tile([P, 1], F32)
        nc.gpsimd.memset(quart, 0.25)
        b1_sb = const.tile([1, AH * C], BF)
        nc.scalar.dma_start(b1_sb, b1[None, :])
        w2t = const.tile([P, 3, C], F8)
        nc.scalar.dma_start(w2t, w2c[:, :, :])
        b2t = const.tile([P, 3], F32)
        nc.scalar.dma_start(b2t, b2c[:, :])
        # output staging: group cb's sigmoid lands in column 32*cb; the
        # block-transpose turns that into rows {0,32,64,96} x 32 cols
        osb32 = const.tile([P, 96], F32)
        nc.gpsimd.memset(osb32, 0.0)
        CB = {g_a: 0, g_b: 1, g_solo: 2}

        # --- input DMAs all issued up front, in consumption order ---
        # x: 5 pair-chunk DMAs
        xts = []
        for kp in range(KC // 2):
            xt = xp.tile([P, 2, B * SP50], F8, tag="xt")
            src = xT[kp * 2 * P:(kp + 1) * 2 * P, :].rearrange(
                "(two p) f -> p two f", p=P)
            nc.sync.dma_start(xt, src)
            xts.append(xt)
        # phase-1 weights: one DMA per k (plain 2D contiguous)
        wat = wga.tile([P, KC, 3, C], F8)
        for k in range(KC):
            nc.sync.dma_start(wat[:, k], wa[:, k])
        # phase-2 weights: 2 DMAs per 512-section, 1 for the 256 tail
        wbt0 = wgb.tile([P, KC, 2, 512], F8, tag="wb0", name="wb0")
        wbt1 = wgb.tile([P, KC, 2, 512], F8, tag="wb1", name="wb1")
        wbt2 = wgb.tile([P, KC, 2, 256], F8, tag="wb2", name="wb2")
        KH = 5
        for ni, (wt, wd) in enumerate(((wbt0, wb0), (wbt1, wb1), (wbt2, wb2))):
            for kh in range(KC // KH):
                nc.sync.dma_start(
                    wt[:, kh * KH:(kh + 1) * KH], wd[:, kh * KH:(kh + 1) * KH])

        # --- stage 1: pooling + h_swish gate -> pT (fp8, 24.5*p) ---
        # gpsimd halves the s-reduction (pairwise add of the two 25-wide
        # halves); DVE reduces the halved tensor. This splits the v1/v2
        # serial DVE pooling chain (~35us at 1x) across two engines.
        pT = st.tile([P, KC, B], F8, tag="pT")
        for k in range(KC):
            xsl = xts[k // 2][:, k % 2, :].rearrange(
                "p (b s) -> p b s", s=SP50)
            th = sp.tile([P, B, 25], BF, tag="th")
            nc.gpsimd.tensor_tensor(
                th, xsl[:, :, 0:25], xsl[:, :, 25:50], ALU.add)
            sums = sp.tile([P, B], F32, tag="sums")
            nc.vector.reduce_sum(sums, th, axis=mybir.AxisListType.X)
            t1 = sp.tile([P, B], F32, tag="t1")
            nc.scalar.activation(t1, sums, AF.Relu, bias=quart, scale=T1_SCALE)
            nc.vector.scalar_tensor_tensor(
                pT[:, k, :], t1, 0.5, sums, ALU.min, ALU.mult)

        # --- PSUM tiles ---
        # phase1: 6 live tiles; phase2 reuses the solo group's tags
        def psum_tiles(base):
            ts = []
            for ni, (n0, nn) in enumerate(NS):
                ts.append(pp.tile([P, nn], F32, tag=f"{base}{ni}",
                                  name=f"{base}{ni}"))
            return ts

        ps_solo = psum_tiles("pss")
        ps_a = psum_tiles("psa")

        def bias_mm(ps, j, a, n0, nn):
            tp = (0, 64 * j) if j else None
            nc.tensor.matmul(
                ps[64 * j:64 * j + B, :], ones,
                b1_sb[:, a * C + n0:a * C + n0 + nn],
                start=False, stop=False, tile_position=tp,
                skip_group_check=True)

        # --- phase 1: heads (4 | 0,1) k-major interleaved ---
        for k in range(KC):
            for ni, (n0, nn) in enumerate(NS):
                # solo head 4: psum partitions 0-63, PE cols 0-63
                nc.tensor.matmul(
                    ps_solo[ni][:B, :], pT[:, k, :], wat[:, k, 0, n0:n0 + nn],
                    start=(k == 0), stop=(k == KC - 1),
                    skip_group_check=True)
            for j in (0, 1):  # pair heads 0,1
                tp = (0, 64 * j) if j else None
                for ni, (n0, nn) in enumerate(NS):
                    nc.tensor.matmul(
                        ps_a[ni][64 * j:64 * j + B, :], pT[:, k, :],
                        wat[:, k, 1 + j, n0:n0 + nn],
                        start=(k == 0), stop=(k == KC - 1),
                        tile_position=tp, skip_group_check=True)
            if k == 0:
                for ni, (n0, nn) in enumerate(NS):
                    bias_mm(ps_solo[ni], 0, 4, n0, nn)
                for j, a in enumerate(g_a):
                    for ni, (n0, nn) in enumerate(NS):
                        bias_mm(ps_a[ni], j, a, n0, nn)

        # --- evacuation helper ---
        def evac(hs, pss):
            rpart = st.tile([P, len(NS)], F32, tag=f"rp{hs[0]}")
            for ni, (n0, nn) in enumerate(NS):
                evac_chunk(hs, pss, ni, n0, nn, rpart)
            evac_fin(hs, rpart)

        def evac_chunk(hs, pss, ni, n0, nn, rpart):
            pn = B * len(hs)
            zs = pss[ni][:pn]
            t1h = ep.tile([P, 512], F32, tag="t1h")
            nc.scalar.activation(
                t1h[:pn, :nn], zs, AF.Relu, bias=half[:pn], scale=T1H_SCALE)
            t2w = ep.tile([P, 512], BF, tag="t2w")
            nc.vector.scalar_tensor_tensor(
                t2w[:pn, :nn], t1h[:pn, :nn], 1.0,
                w2t[:pn, CB[hs], n0:n0 + nn], ALU.min, ALU.mult)
            scr = ep.tile([P, 512], F32, tag="scr")
            nc.vector.scalar_tensor_tensor(
                scr[:pn, :nn], zs, SCR_SCALE, t2w[:pn, :nn],
                ALU.mult, ALU.mult, accum_out=rpart[:pn, ni:ni + 1])

        def evac_fin(hs, rpart):
            pn = B * len(hs)
            cb = CB[hs]
            rlog = st.tile([P, 1], F32, tag=f"rl{hs[0]}")
            nc.vector.reduce_sum(rlog[:pn], rpart[:pn, :],
                                 axis=mybir.AxisListType.X)
            nc.scalar.activation(osb32[:pn, 32 * cb:32 * cb + 1], rlog[:pn],
                                 AF.Sigmoid, bias=b2t[:pn, cb:cb + 1])

        evac(g_solo, ps_solo)
        evac(g_a, ps_a)

        # --- phase 2: pair (2,3), n-major stream; psum reuses solo tags ---
        ps_b = psum_tiles("pss")
        rpart_b = st.tile([P, len(NS)], F32, tag="rp2")
        for ni, (n0, nn) in enumerate(NS):
            wt = (wbt0, wbt1, wbt2)[ni]
            for j in (0, 1):
                tp = (0, 64 * j) if j else None
                for k in range(KC):
                    nc.tensor.matmul(
                        ps_b[ni][64 * j:64 * j + B, :], pT[:, k, :],
                        wt[:, k, j, :nn],
                        start=(k == 0), stop=(k == KC - 1),
                        tile_position=tp, skip_group_check=True)
                    if k == 0:
                        bias_mm(ps_b[ni], j, g_b[j], n0, nn)
            evac_chunk(g_b, ps_b, ni, n0, nn, rpart_b)

        evac_fin(g_b, rpart_b)

        # --- single contiguous output store ---
        # block-transpose [128, 96] puts group cb's 128 results on rows
        # {0,32,64,96} cols [32cb, 32cb+32); one DMA writes all of them
        tr = st.tile([P, 96], F32, tag="tr")
        nc.vector.transpose(tr, osb32)
        src = tr[0:P:32, :].rearrange("q (cb j) -> q cb j", j=32)
        dst = out[:].rearrange("(cb q j) -> q cb j", q=4, j=32)
        nc.sync.dma_start(dst, src)

    nc.compile()
    return nc


def get_nc():
    if "nc" not in _NC_CACHE:
        _NC_CACHE["nc"] = build_nc()
    return _NC_CACHE["nc"]


def make_in_maps(x, W1, b1, W2, b2):
    f8 = ml_dtypes.float8_e3m4
    bf = ml_dtypes.bfloat16
    x = np.asarray(x, dtype=np.float32)
    W1 = np.asarray(W1, dtype=np.float32)
    b1 = np.asarray(b1, dtype=np.float32)
    W2 = np.asarray(W2, dtype=np.float32)
    b2 = np.asarray(b2, dtype=np.float32)

    # [B, C, H, W] -> [C, B*50] (s zero-padded), replicated to all cores
    xp50 = np.zeros((C, B, SP50), np.float32)
    xp50[:, :, :S] = x.reshape(B, C, S).transpose(1, 0, 2)
    xT = xp50.reshape(C, B * SP50).astype(f8)

    in_maps = []
    for core in range(NCORES):
        a0 = core * AH
        # per-head k-chunked layout: wh[a][k, p, d] = W1[a0+a][k*128+p, d]
        whs = (W1[a0:a0 + AH] * SC_W1).reshape(AH, KC, P, C)
        # phase-1 stream [P, KC, 3, C], heads (4, 0, 1)
        wa = np.empty((P, KC, 3, C), np.float32)
        for jj, a in enumerate(PH1):
            wa[:, :, jj, :] = whs[a].transpose(1, 0, 2)
        # phase-2 streams [P, KC, 2, nn] per n-section, heads (2, 3)
        wbs = []
        for n0, nn in NS:
            wbn = np.empty((P, KC, 2, nn), np.float32)
            for jj, a in enumerate(PH2):
                wbn[:, :, jj, :] = whs[a][:, :, n0:n0 + nn].transpose(1, 0, 2)
            wbs.append(wbn)
        w2s = W2[a0:a0 + AH] * SC_W2  # [AH, C]
        # w2c[r, cb, :] = W2[GORDER[cb][r // 64]]; b2c analogous
        w2c = np.zeros((P, 3, C), np.float32)
        b2c = np.zeros((P, 3), np.float32)
        for cb, hs in enumerate(GORDER):
            for j, a in enumerate(hs):
                w2c[64 * j:64 * (j + 1), cb, :] = w2s[a]
                b2c[64 * j:64 * (j + 1), cb] = b2[a0 + a]
        in_maps.append({
            "xT": xT,
            "wa": np.ascontiguousarray(wa).astype(f8),
            "wb0": np.ascontiguousarray(wbs[0]).astype(f8),
            "wb1": np.ascontiguousarray(wbs[1]).astype(f8),
            "wb2": np.ascontiguousarray(wbs[2]).astype(f8),
            "b1": np.ascontiguousarray(b1[a0:a0 + AH] * SC_B1
                                       ).reshape(AH * C).astype(bf),
            "w2c": w2c.astype(f8),
            "b2c": b2c,
        })
    return in_maps


def kernel(x, W1, b1, W2, b2, _trace=False, _tmpdir=None):
    from concourse.bass_utils import run_bass_kernel_spmd

    nc = get_nc()
    in_maps = make_in_maps(x, W1, b1, W2, b2)
    res = run_bass_kernel_spmd(
        nc, in_maps, core_ids=list(range(NCORES)),
        trace=_trace, tmpdir=_tmpdir,
    )
    # out flat is (cb, head_within, batch) with GORDER head mapping and a
    # pad row; reassemble to [B, AH] per core
    outs = []
    for c in range(NCORES):
        o = np.asarray(res.results[c]["out"], dtype=np.float32).reshape(6, B)
        oc = np.empty((B, AH), np.float32)
        for cb, hs in enumerate(GORDER):
            for j, a in enumerate(hs):
                oc[:, a] = o[2 * cb + j]
        outs.append(oc)
    full = np.concatenate(outs, axis=1)  # [B, A]
    if _trace:
        return full, res
    return full
